# revision 98
# baseline (speedup 1.0000x reference)
"""Frame-causal sparse attention block (LN + QKV + masked softmax attention
+ out-proj) on 8 TRN2 NeuronCores.

Sharding: core c handles batch b = c//2 and heads [4*(c%2), 4*(c%2)+4);
the out-projection is column-split (each core of a pair produces 256 of
the 512 output columns for all tokens; the host concatenates).

Per-core, fully on-chip, a software pipeline over four 512-query chunks:

  prologue: x as 16 contiguous 128KB DMA slabs; LN stats via PE
    inv512-column matmuls; rank-1 -mu (x) csg matmul replaces mean
    centering (host verifies the ln_b column-sum csb == 0); rstd
    partition-broadcast via a PE ones-row outer product; rstd scatter to
    [token%128, tile] bounces through DRAM for the per-partition V scale.
  round j: attention of chunk j for both head-pairs: block-sparse
    S^T = K @ Q^T (two 64-contract matmuls packed in the PE via row
    tile_position, 3-deep lookahead), exp on ScalarE (scale 1/8),
    exact-width A@V accumulation; QKV of chunk j+1 is emitted around
    p1's S^T pipeline so the exp stream never waits on it; softmax
    denominators (ones columns inside the V blocks at row 64 / row 32)
    are gathered [2, 512] per pair via ScalarE-issued DMAs and a Ln/Exp
    reciprocal; RAW attention output + reciprocal rows AllGather to the
    pair peer immediately (latency hides under later rounds); both rank
    blocks are normalized locally (selector-matmul broadcast of 1/D) and
    out-projected one round later into this core's 256 output columns.

DMA streams: bulk traffic and collective recvs on the SP HWDGE,
latency-critical D-row gathers on the ACT HWDGE, collective triggers on
gpsimd (software DGE is ~4us/DMA — never put real transfers there).
"""

import sys

import numpy as np

sys.path.insert(0, "/opt/trn_rl_repo")

DIM = 512
HEADS = 8
DH = 64
INNER = 512
T = 2048
B = 4
EPS = 1e-5
NCORES = 8
HPC = 4  # heads per core
CQ = HPC * DH  # 256 channels per core for each of Q, K, V
NT = T // 128  # 16 token tiles

_cache = {}


def _build_nc(stage=4):
    from contextlib import ExitStack

    import concourse.bacc as bacc
    import concourse.bass as bass
    import concourse.tile as tile
    from concourse import mybir

    f32 = mybir.dt.float32
    bf16 = mybir.dt.bfloat16
    AF = mybir.ActivationFunctionType
    OP = mybir.AluOpType

    # Route every Exp/Ln activation to the one table set that contains both
    # (natural_log_exp_and_others): the default first-match pick splits them
    # across two sets and the per-division ln/exp chain then reloads ACT
    # tables repeatedly (~2.7us each).
    if not getattr(bacc, "_act_tables_patched", False):
        _orig_get_tables = bacc.get_activation_tables

        def _patched_get_tables(arch):
            tabs = _orig_get_tables(arch)
            both = [
                n
                for n, fns in tabs.items()
                if mybir.ActivationFunctionType.Exp in fns
                and mybir.ActivationFunctionType.Ln in fns
            ]
            if both:
                keep = both[0]
                tabs = {
                    n: (
                        fns
                        if n == keep
                        else fns
                        - {
                            mybir.ActivationFunctionType.Exp,
                            mybir.ActivationFunctionType.Ln,
                        }
                    )
                    for n, fns in tabs.items()
                }
            return tabs

        bacc.get_activation_tables = _patched_get_tables
        bacc._act_tables_patched = True

    nc = bacc.Bacc(
        "TRN2",
        target_bir_lowering=False,
        debug=False,
        num_devices=NCORES,
    )

    # ---- external I/O ----
    x_t = nc.dram_tensor("x_t", [DIM, T], bf16, kind="ExternalInput")
    # [512, 768] = [W'_q(256) | W'_k(256) | W'_v(256)] with LN-g folded in,
    # Q/K/V column blocks each ordered [h0|h1|h2|h3] x 64
    w_qkv_s = nc.dram_tensor("w_qkv_s", [DIM, 3 * CQ], bf16, kind="ExternalInput")
    # column sums of the g-folded weights (rank-1 mean correction)
    cs1 = nc.dram_tensor("cs1", [1, 3 * CQ], bf16, kind="ExternalInput")
    # all 512 w_out rows (head order h0..h7 x 64), this core's 256 out-columns
    w_out_s = nc.dram_tensor("w_out_s", [4 * 128, 256], bf16, kind="ExternalInput")
    b_half = nc.dram_tensor("b_half", [1, 256], bf16, kind="ExternalInput")
    # selector for broadcasting the 2 reciprocal rows to 64 partitions each
    sel2 = nc.dram_tensor("sel2", [2, 128], bf16, kind="ExternalInput")
    out_ext = nc.dram_tensor("out", [T, DIM], bf16, kind="ExternalOutput")

    with tile.TileContext(nc) as tc:
        with ExitStack() as stack:
            constp = stack.enter_context(tc.tile_pool(name="const", bufs=1))
            statp = stack.enter_context(tc.tile_pool(name="statp", bufs=2))
            work = stack.enter_context(tc.tile_pool(name="work", bufs=2))
            epool = stack.enter_context(tc.tile_pool(name="epool", bufs=4))
            opool = stack.enter_context(tc.tile_pool(name="opool", bufs=4))
            dpool = stack.enter_context(tc.tile_pool(name="dpool", bufs=4))
            rpool = stack.enter_context(tc.tile_pool(name="rpool", bufs=2))
            gpool = stack.enter_context(tc.tile_pool(name="gpool", bufs=2))
            ps_s = stack.enter_context(tc.tile_pool(name="ps_s", bufs=2, space="PSUM"))
            ps_o = stack.enter_context(tc.tile_pool(name="ps_o", bufs=1, space="PSUM"))
            ps_m = stack.enter_context(tc.tile_pool(name="ps_m", bufs=2, space="PSUM"))
            dram = stack.enter_context(tc.tile_pool(name="dram", bufs=1, space="DRAM"))
            xpool = stack.enter_context(tc.tile_pool(name="xpool", bufs=1))
            pers = stack.enter_context(tc.tile_pool(name="pers", bufs=1))

            # x as 16 fully-contiguous 128KB slabs (strided chunk loads run at
            # ~1/3 bandwidth); everything lands by ~8us
            xw = [
                xpool.tile([128, T], bf16, name=f"x{d}", tag=f"x{d}") for d in range(4)
            ]
            for d in range(4):
                for r in range(4):
                    nc.sync.dma_start(
                        xw[d][32 * r : 32 * (r + 1), :],
                        x_t[d * 128 + 32 * r : d * 128 + 32 * (r + 1), :],
                    )

            # ---------- constants / weights ----------
            w_sb = [
                constp.tile([128, 3 * CQ], bf16, name=f"w{d}", tag=f"w{d}")
                for d in range(4)
            ]
            for d in range(4):
                nc.sync.dma_start(w_sb[d][:], w_qkv_s[d * 128 : (d + 1) * 128, :])
            ones_row = constp.tile([1, 128], bf16)
            nc.vector.memset(ones_row[:], 1.0)
            inv512 = constp.tile([128, 1], bf16)
            nc.vector.memset(inv512[:], 1.0 / DIM)
            eps_col = constp.tile([1, 1], f32)
            nc.vector.memset(eps_col[:], EPS)
            rstd_colb = constp.tile([128, NT], bf16)
            rstd_col = constp.tile([128, NT], f32)

            # needed by the first corr matmul (~10us in): before wout
            csg_sb = constp.tile([1, 3 * CQ], bf16)
            nc.sync.dma_start(csg_sb[:], cs1[:])
            # late-needed weights after the x stream
            wout_sb = [
                constp.tile([128, 256], bf16, name=f"wo{g}", tag=f"wo{g}")
                for g in range(4)
            ]
            for g in range(4):
                nc.sync.dma_start(wout_sb[g][:], w_out_s[g * 128 : (g + 1) * 128, :])
            brep = constp.tile([128, 256], bf16)
            nc.sync.dma_start(brep[:], b_half[:].broadcast_to((128, 256)))
            sel_sb = constp.tile([2, 128], bf16)
            nc.sync.dma_start(sel_sb[:], sel2[:])

            # persistent intermediates
            rstd_rep = pers.tile([128, T], bf16, name="rstd_rep", tag="rstd_rep")
            corr2 = pers.tile([1, T], bf16, name="corr2", tag="corr2")
            qT = [
                pers.tile([128, T], bf16, name=f"qT{p}", tag=f"qT{p}")
                for p in range(2)
            ]
            kT = [
                pers.tile([128, T], bf16, name=f"kT{p}", tag=f"kT{p}")
                for p in range(2)
            ]
            # V per k-tile: 4 blocks of 128 cols; even head block = [V(64) |
            # ones@64 | 0...], odd head block = [0... | ones@32 | 0 | V@64:128]
            v_sb = pers.tile([128, NT * 512], bf16, name="v_sb", tag="v_sb")

            # DRAM bounce tensors
            rstd_dram = dram.tile([1, T], bf16, name="rstd_dram", tag="rstd_dram")
            rstdT_dram = dram.tile([128, NT], bf16, name="rstdT", tag="rstdT")
            # per (pair, chunk) AllGather of RAW attention output + reciprocal
            # rows, fired immediately after each pair's A@V so the collective
            # latency hides under subsequent compute; each core divides the
            # peer's heads locally and out-projects into its own 256 columns
            ag_send = [
                [
                    dram.tile([130, 512], bf16, name=f"as{p}_{j}", tag=f"as{p}_{j}")
                    for j in range(4)
                ]
                for p in range(2)
            ]
            ag_recv = [
                [
                    dram.tile([260, 512], bf16, name=f"ar{p}_{j}", tag=f"ar{p}_{j}")
                    for j in range(4)
                ]
                for p in range(2)
            ]

            # ---------- prologue: LN statistics ----------
            rstd_rows = {}

            def emit_rep(j):
                # partition-broadcast on the PE (ones-row outer product);
                # emitted after ALL stats so the ps_m slot chain never makes
                # a later chunk's stats wait on this chunk's ACT rstd
                # borrow the (still idle) attention S^T PSUM so neither the
                # stats nor the QKV ps_m slot chain couples to the ACT rstd
                cl = slice(j * 512, (j + 1) * 512)
                rep_ps = ps_s.tile([128, 1024], f32, name="rep_ps", tag="s_ps")
                nc.tensor.matmul(rep_ps[:, 0:512], ones_row[:], rstd_rows[j][:])
                nc.vector.tensor_copy(rstd_rep[:, cl], rep_ps[:, 0:512])

            def emit_stats(j):
                cl = slice(j * 512, (j + 1) * 512)
                s1t = ps_m.tile([128, 512], f32, name="s1t", tag="m")
                s2t = ps_m.tile([128, 512], f32, name="s2t", tag="m")
                s1 = s1t[0:1, :]
                s2 = s2t[0:1, :]
                for d in range(4):
                    nc.tensor.matmul(
                        s1, inv512[:], xw[d][:, cl], start=(d == 0), stop=(d == 3)
                    )
                for d in range(4):
                    xsq = work.tile([128, 512], bf16, name="xsq", tag="xsq")
                    nc.vector.tensor_tensor(xsq[:], xw[d][:, cl], xw[d][:, cl], OP.mult)
                    nc.tensor.matmul(
                        s2, inv512[:], xsq[:], start=(d == 0), stop=(d == 3)
                    )
                # corr2 = -mu (rank-1 mean correction row); var from E[x^2]-mu^2
                nc.vector.tensor_scalar(corr2[0:1, cl], s1, -1.0, None, OP.mult)
                musq = statp.tile([1, 512], f32, name="musq", tag="musq")
                nc.vector.tensor_tensor(
                    musq[:], corr2[0:1, cl], corr2[0:1, cl], OP.mult
                )
                var = statp.tile([1, 512], f32, name="var", tag="var")
                nc.vector.tensor_tensor(var[:], s2, musq[:], OP.subtract)
                lnv = statp.tile([1, 512], f32, name="lnv", tag="lnv")
                nc.scalar.activation(lnv[:], var[:], AF.Ln, bias=eps_col[:], scale=1.0)
                rstd_row = statp.tile(
                    [1, 512], bf16, name="rstd_row", tag="rstd_row", bufs=4
                )
                nc.scalar.activation(rstd_row[:], lnv[:], AF.Exp, bias=0.0, scale=-0.5)
                rstd_rows[j] = rstd_row
                # scatter rstd into [token%128, tile] layout for the V scaling
                nc.sync.dma_start(
                    rstdT_dram[:, 4 * j : 4 * j + 4].rearrange("p t -> t p"),
                    rstd_row[:].rearrange("o (t p) -> o t p", p=128),
                )
                nc.sync.dma_start(
                    rstd_colb[:, 4 * j : 4 * j + 4],
                    rstdT_dram[:, 4 * j : 4 * j + 4],
                )

            def emit_rstdcol(j):
                # deferred: this copy waits a 2-hop DMA bounce, so inside the
                # stats chain it would serialize each chunk by ~8us
                nc.vector.tensor_copy(
                    rstd_col[:, 4 * j : 4 * j + 4], rstd_colb[:, 4 * j : 4 * j + 4]
                )

            # ---------- emission helpers ----------
            def emit_qkv_block(j, ct):
                # ct 0,1 -> qT pairs; 2,3 -> kT pairs
                cl = slice(j * 512, (j + 1) * 512)
                dst = qT[ct] if ct < 2 else kT[ct - 2]
                wcl = slice(ct * 128, (ct + 1) * 128)
                acc = ps_m.tile([128, 512], f32, name="qkv_ps", tag="m")
                for d in range(4):
                    nc.tensor.matmul(
                        acc[:],
                        w_sb[d][:, wcl],
                        xw[d][:, cl],
                        start=(d == 0),
                        stop=False,
                    )
                nc.tensor.matmul(
                    acc[:], csg_sb[0:1, wcl], corr2[0:1, cl], start=False, stop=True
                )
                nc.vector.tensor_tensor(dst[:, cl], acc[:], rstd_rep[:, cl], OP.mult)

            def emit_v(tt):
                tl = slice(tt * 128, (tt + 1) * 128)
                vact = ps_m.tile([128, 512], f32, name="v_ps", tag="m")
                vac = vact[:, 0:CQ]
                for d in range(4):
                    nc.tensor.matmul(
                        vac,
                        xw[d][:, tl],
                        w_sb[d][:, 2 * CQ : 3 * CQ],
                        start=(d == 0),
                        stop=False,
                    )
                nc.tensor.matmul(
                    vac,
                    corr2[0:1, tl],
                    csg_sb[0:1, 2 * CQ : 3 * CQ],
                    start=False,
                    stop=True,
                )
                blk = v_sb[:, tt * 512 : (tt + 1) * 512].rearrange(
                    "p (a b) -> p a b", b=256
                )
                src = vac.rearrange("p (a b) -> p a b", b=128)
                rc = rstd_col[:, tt : tt + 1]
                nc.vector.tensor_scalar(
                    blk[:, :, 0:64], src[:, :, 0:64], rc, None, OP.mult
                )
                nc.vector.tensor_scalar(
                    blk[:, :, 192:256], src[:, :, 64:128], rc, None, OP.mult
                )

            def attn_start(j, p):
                # allocate PSUM and emit the first two S^T tiles early so the
                # exp stream restarts as soon as the chunk's qT lands
                nkt = 4 * (j + 1)

                def st(i):
                    q0 = max(512 * j, 128 * i)
                    n = 512 * (j + 1) - q0
                    off = q0 - 512 * j
                    s_ps = ps_s.tile([128, 1024], f32, name="s_ps", tag="s_ps")
                    for h in range(2):
                        hr = slice(h * 64, (h + 1) * 64)
                        nc.tensor.matmul(
                            s_ps[:, h * 512 + off : h * 512 + off + n],
                            kT[p][hr, i * 128 : (i + 1) * 128],
                            qT[p][hr, q0 : q0 + n],
                        )
                    return s_ps, off, n

                o_ps = [
                    ps_o.tile([128, 512], f32, name=f"o_ps{h}", tag=f"o_ps{h}")
                    for h in range(2)
                ]
                ctx = dict(j=j, p=p, st=st, o_ps=o_ps, pend={}, nst=0)

                def more():
                    # pre-emit one more S^T tile (slot aliasing is safe; the
                    # scheduler orders it after the aliased slot's readers)
                    if ctx["nst"] < 4 * (j + 1):
                        ctx["pend"][ctx["nst"]] = st(ctx["nst"])
                        ctx["nst"] += 1

                ctx["more"] = more
                more()
                more()
                return ctx

            def attn_finish(ctx, inject=None):
                j, p, st, o_ps, pend = (
                    ctx["j"],
                    ctx["p"],
                    ctx["st"],
                    ctx["o_ps"],
                    ctx["pend"],
                )
                nkt = 4 * (j + 1)
                for i in range(nkt):
                    s_ps, off, n = pend.pop(i)
                    e_sb = epool.tile([128, 1024], bf16, name="e_sb", tag="e_sb")
                    sr = s_ps[:].rearrange("p (x n) -> p x n", x=2)[:, :, off : off + n]
                    er = e_sb[:].rearrange("p (x n) -> p x n", x=2)[:, :, off : off + n]
                    nc.scalar.activation(er, sr, AF.Exp, bias=0.0, scale=0.125)
                    if stage == 2 and j == 0 and p == 0 and i == 0:
                        nc.sync.dma_start(out_ext[256:384, :], e_sb[:, 0:512])
                    if i == 1 and inject is not None:
                        inject()
                    if i >= 4 * j:
                        # frame-causal quadrant: rows 64:128 (frame 2i+1) must
                        # not be seen by queries 128i..128i+64
                        ez = e_sb[64:128, :].rearrange("p (x n) -> p x n", x=2)[
                            :, :, off : off + 64
                        ]
                        nc.vector.memset(ez, 0.0)
                    first = i == 0
                    last = i == nkt - 1
                    for h in range(2):
                        g = 2 * p + h
                        nc.tensor.matmul(
                            o_ps[h][:, off:512],
                            v_sb[:, i * 512 + g * 128 : i * 512 + (g + 1) * 128],
                            e_sb[:, h * 512 + off : (h + 1) * 512],
                            start=first,
                            stop=last,
                            skip_group_check=True,
                        )
                    while ctx["nst"] < min(i + 3, nkt):
                        ctx["more"]()
                # stash raw O (bf16) and denominator rows; SBUF->SBUF DMA
                # batches both heads' D rows onto partitions 0-1
                oraw = opool.tile([128, 512], bf16, name="oraw", tag="oraw")
                or2 = opool.tile([65, 512], bf16, name="or2", tag="or2")
                nc.vector.tensor_copy(oraw[0:64, :], o_ps[0][0:64, :])
                nc.vector.tensor_copy(oraw[64:128, :], o_ps[1][64:128, :])
                nc.vector.tensor_copy(or2[64:65, :], o_ps[0][64:65, :])
                nc.vector.tensor_copy(or2[32:33, :], o_ps[1][32:33, :])
                dstt = dpool.tile([2, 512], bf16, name="dstt", tag="dstt")
                nc.scalar.dma_start(dstt[0:1, :], or2[64:65, :])
                nc.scalar.dma_start(dstt[1:2, :], or2[32:33, :])
                if stage == 2 and j == 0:
                    nc.sync.dma_start(out_ext[128 * p : 128 * (p + 1), :], oraw[:])
                return oraw, dstt

            def emit_recip(p, j, dst):
                # batched 1/D for both heads: exp(-ln(D))
                lnd = dpool.tile([2, 512], f32, name="lnd", tag="lnd")
                nc.scalar.activation(lnd[:], dst[:], AF.Ln, bias=0.0, scale=1.0)
                rec = dpool.tile([2, 512], bf16, name="rec", tag="rec")
                nc.scalar.activation(rec[:], lnd[:], AF.Exp, bias=0.0, scale=-1.0)
                if stage == 2 and j == 0:
                    nc.sync.dma_start(out_ext[384 + 2 * p : 386 + 2 * p, :], dst[:])
                    nc.sync.dma_start(out_ext[388 + 2 * p : 390 + 2 * p, :], rec[:])
                return rec

            def emit_selmm(rec):
                rrep_ps = ps_m.tile([128, 512], f32, name="rrep_ps", tag="m")
                nc.tensor.matmul(rrep_ps[:], sel_sb[:], rec[:])
                return rrep_ps

            def emit_div(p, j, oraw, rrep_ps):
                cl = slice(j * 512, (j + 1) * 512)
                rrep = rpool.tile([128, 512], bf16, name="rrep", tag="rrep")
                nc.vector.tensor_copy(rrep[:], rrep_ps[:])
                nc.vector.tensor_tensor(onormP[p][:, cl], oraw[:], rrep[:], OP.mult)

            def emit_ag(p, j, oraw, rec):
                # send this pair's raw O + reciprocal rows to the pair peer
                nc.sync.dma_start(ag_send[p][j][0:128, :], oraw[:])
                nc.sync.dma_start(ag_send[p][j][128:130, :], rec[:])
                nc.gpsimd.collective_compute(
                    "AllGather",
                    OP.bypass,
                    replica_groups=[[2 * b, 2 * b + 1] for b in range(B)],
                    ins=[ag_send[p][j][:].opt()],
                    outs=[ag_recv[p][j][:].opt()],
                )
                # AllGather output is rank-ordered, so both rank blocks are
                # fetched and normalized locally (SPMD-uniform)
                out = []
                for r in range(2):
                    po = gpool.tile(
                        [128, 512], bf16, name=f"po{p}{r}", tag=f"po{p}{r}"
                    )
                    pr = gpool.tile([2, 512], bf16, name=f"pc{p}{r}", tag=f"pc{p}{r}")
                    nc.sync.dma_start(po[:], ag_recv[p][j][130 * r : 130 * r + 128])
                    nc.sync.dma_start(
                        pr[:], ag_recv[p][j][130 * r + 128 : 130 * r + 130]
                    )
                    out.append((po, pr))
                return out  # [(rawO, rec) for rank 0, rank 1]

            def emit_norm(po, pr):
                # normalize one gathered head-pair block
                rrep_ps = emit_selmm(pr)
                rrep = rpool.tile([128, 512], bf16, name="prr", tag="prr")
                nc.vector.tensor_copy(rrep[:], rrep_ps[:])
                pn = gpool.tile([128, 512], bf16, name="pn", tag="pn", bufs=4)
                nc.vector.tensor_tensor(pn[:], po[:], rrep[:], OP.mult)
                return pn

            def emit_outproj(tt, pns):
                # pns[g] = normalized [128, 512] chunk block, g = 2*rank + p
                # (natural head order h0..h7); w_out rows natural on all cores
                tl = slice(tt * 128, (tt + 1) * 128)
                lc = slice((tt % 4) * 128, (tt % 4 + 1) * 128)
                ops = ps_m.tile([128, 512], f32, name="out_ps", tag="m")
                for g in range(4):
                    nc.tensor.matmul(
                        ops[:, 0:256],
                        pns[g][:, lc],
                        wout_sb[g][:],
                        start=(g == 0),
                        stop=(g == 3),
                    )
                pst = work.tile([128, 256], bf16, name="pst", tag="pst")
                nc.vector.tensor_tensor(pst[:], ops[:, 0:256], brep[:], OP.add)
                nc.sync.dma_start(out_ext[tl, 0:256], pst[:])

            # ---------- pipelined rounds ----------
            # QKV for chunk j+1 is emitted inside round j, so each round's
            # attention starts on data prepared one round earlier
            agst = {}  # (p, j) -> [(rawO, rec) per rank]

            def emit_norms(jj):
                # normalize chunk jj's gathered blocks (g = 2*rank + p)
                ag0, ag1 = agst.pop((0, jj)), agst.pop((1, jj))
                pns = [
                    emit_norm(*ag0[0]),
                    emit_norm(*ag1[0]),
                    emit_norm(*ag0[1]),
                    emit_norm(*ag1[1]),
                ]
                if stage == 3 and jj in (0, 3):
                    ro = 0 if jj == 0 else 256
                    nc.sync.dma_start(out_ext[ro : ro + 128, :], pns[0][:])
                    nc.sync.dma_start(out_ext[ro + 128 : ro + 256, :], pns[1][:])
                return pns

            # prologue: only what round-0 attention needs (chunks 2/3 stats
            # would head-block the PE on their x DMAs; they move to round 0)
            emit_stats(0)
            emit_stats(1)
            emit_stats(2)
            emit_stats(3)
            for j in range(4):
                emit_rep(j)
            emit_qkv_block(0, 0)
            emit_qkv_block(0, 2)
            emit_qkv_block(0, 1)
            emit_qkv_block(0, 3)
            for j in range(4):
                emit_rstdcol(j)
            # V staging memsets precede the first emit_v in DVE order
            nc.vector.memset(v_sb[:], 0.0)
            vview = v_sb[:].rearrange("p (t a b) -> p t a b", t=NT, b=256)
            nc.vector.memset(vview[:, :, :, 64:65], 1.0)
            nc.vector.memset(vview[:, :, :, 160:161], 1.0)
            for tt in range(4):
                emit_v(tt)
            a0 = attn_start(0, 0) if stage >= 2 else None
            for j in range(4):
                if stage >= 2:
                    oraw0, dst0 = attn_finish(a0)

                    def inj1(jj=j, o=oraw0, d=dst0):
                        rec = emit_recip(0, jj, d)
                        if stage >= 3:
                            agst[(0, jj)] = emit_ag(0, jj, o, rec)

                    # p1's S^T pipeline starts before next chunk's QKV so the
                    # exp stream never waits behind the QKV matmul block
                    a1 = attn_start(j, 1)
                    if j < 3:
                        emit_qkv_block(j + 1, 0)
                        a1["more"]()
                        emit_qkv_block(j + 1, 2)
                        a1["more"]()
                    oraw1, dst1 = attn_finish(a1, inject=inj1)
                    if j < 3:
                        emit_qkv_block(j + 1, 1)
                        emit_qkv_block(j + 1, 3)
                        for tt in range(4 * (j + 1), 4 * (j + 1) + 4):
                            emit_v(tt)
                    rec1 = emit_recip(1, j, dst1)
                    if stage >= 3:
                        agst[(1, j)] = emit_ag(1, j, oraw1, rec1)
                    if j < 3:
                        a0 = attn_start(j + 1, 0)
                    if j > 0 and stage >= 3:
                        pns = emit_norms(j - 1)
                        if stage >= 4:
                            for tt in range(4 * (j - 1), 4 * j):
                                emit_outproj(tt, pns)
                elif j < 3:
                    emit_qkv_block(j + 1, 0)
                    emit_qkv_block(j + 1, 2)
                    emit_qkv_block(j + 1, 1)
                    emit_qkv_block(j + 1, 3)
                    for tt in range(4 * (j + 1), 4 * (j + 1) + 4):
                        emit_v(tt)
            # tail: last chunk's gathers, normalize, out-proj
            if stage >= 3:
                pns = emit_norms(3)
            if stage >= 4:
                for tt in range(12, 16):
                    emit_outproj(tt, pns)

            # ---------- debug dumps ----------
            if stage == 1:
                nc.sync.dma_start(out_ext[0:128, :], qT[0][:, 0:512])
                nc.sync.dma_start(out_ext[128:256, :], kT[0][:, 0:512])
                nc.sync.dma_start(out_ext[256:384, :], v_sb[:, 0:512])
                nc.sync.dma_start(out_ext[384:512, :], rstd_rep[:, 0:512])
            if stage == 2:
                nc.sync.dma_start(out_ext[400:528, :], v_sb[:, 1024:1536])
                nc.sync.dma_start(out_ext[528:656, :], v_sb[:, 1536:2048])
                nc.sync.dma_start(out_ext[656:784, 0:NT], rstd_colb[:])
            if stage == 3:
                nc.sync.dma_start(out_ext[0:128, :], onormP[0][:, 0:512])
                nc.sync.dma_start(out_ext[128:256, :], onormP[1][:, 0:512])
                nc.sync.dma_start(out_ext[256:384, :], onormP[0][:, 1536:2048])
                nc.sync.dma_start(out_ext[384:512, :], onormP[1][:, 1536:2048])

    nc.compile()
    return nc


def _prep_in_maps(x, ln_g, ln_b, w_qkv, w_out, b_out):
    import ml_dtypes

    bf = ml_dtypes.bfloat16
    wp = ln_g[:, None] * w_qkv  # [512, 1536]
    csb = (ln_b[:, None] * w_qkv).sum(axis=0)
    assert np.abs(csb).max() == 0.0, "nonzero ln_b not supported by this build"
    sel = np.zeros((2, 128), dtype=np.float32)
    sel[0, :64] = 1.0
    sel[1, 64:] = 1.0
    in_maps = []
    for c in range(NCORES):
        b = c // 2
        heads = range(4 * (c % 2), 4 * (c % 2) + 4)
        qcols = np.concatenate([np.arange(h * DH, (h + 1) * DH) for h in heads])
        cols = np.concatenate([qcols, INNER + qcols, 2 * INNER + qcols])
        ocols = slice(0, 256) if c % 2 == 0 else slice(256, 512)
        in_maps.append(
            {
                "x_t": np.ascontiguousarray(x[b].T).astype(bf),
                "w_qkv_s": np.ascontiguousarray(wp[:, cols]).astype(bf),
                "cs1": np.ascontiguousarray(wp[:, cols].sum(axis=0))
                .reshape(1, -1)
                .astype(bf),
                "w_out_s": np.ascontiguousarray(w_out[:, ocols]).astype(bf),
                "b_half": np.ascontiguousarray(b_out[ocols]).reshape(1, 256).astype(bf),
                "sel2": sel.astype(bf),
            }
        )
    return in_maps


def _run(inputs, trace=False):
    from concourse.bass_utils import run_bass_kernel_spmd

    import os

    stage = int(os.environ.get("KSTAGE", "4"))
    if ("nc", stage) not in _cache:
        _cache[("nc", stage)] = _build_nc(stage)
    nc = _cache[("nc", stage)]
    in_maps = _prep_in_maps(
        np.asarray(inputs["x"], dtype=np.float32),
        np.asarray(inputs["ln_g"], dtype=np.float32),
        np.asarray(inputs["ln_b"], dtype=np.float32),
        np.asarray(inputs["w_qkv"], dtype=np.float32),
        np.asarray(inputs["w_out"], dtype=np.float32),
        np.asarray(inputs["b_out"], dtype=np.float32),
    )
    res = run_bass_kernel_spmd(nc, in_maps, core_ids=list(range(NCORES)), trace=trace)
    out = np.empty((B, T, DIM), dtype=np.float32)
    for b in range(B):
        ev = res.results[2 * b]["out"].astype(np.float32)
        od = res.results[2 * b + 1]["out"].astype(np.float32)
        if stage >= 4:
            # each core computed its own 256 output columns (into cols 0:256)
            out[b][:, 0:256] = ev[:, 0:256]
            out[b][:, 256:512] = od[:, 0:256]
        else:
            out[b] = ev
    return out, res


def kernel(**inputs):
    return _run(inputs, trace=False)[0]


def kernel_traced(**inputs):
    out, res = _run(inputs, trace=True)
    return out, res


# revision 100
# speedup vs baseline: 1.0798x; 1.0798x over previous
"""Frame-causal sparse attention block (LN + QKV + masked softmax attention
+ out-proj) on 8 TRN2 NeuronCores.

Sharding: core c handles batch b = c//2 and heads [4*(c%2), 4*(c%2)+4);
the out-projection is column-split (each core of a pair produces 256 of
the 512 output columns for all tokens; the host concatenates).

Per-core, fully on-chip, a software pipeline over four 512-query chunks:

  prologue: x as 16 contiguous 128KB DMA slabs; LN stats via PE
    inv512-column matmuls; rank-1 -mu (x) csg matmul replaces mean
    centering (host verifies the ln_b column-sum csb == 0); rstd
    partition-broadcast via a PE ones-row outer product; rstd scatter to
    [token%128, tile] bounces through DRAM for the per-partition V scale.
  round j: attention of chunk j for both head-pairs: block-sparse
    S^T = K @ Q^T (two 64-contract matmuls packed in the PE via row
    tile_position, 3-deep lookahead), exp on ScalarE (scale 1/8),
    exact-width A@V accumulation; QKV of chunk j+1 is emitted around
    p1's S^T pipeline so the exp stream never waits on it; softmax
    denominators (ones columns inside the V blocks at row 64 / row 32)
    are gathered [2, 512] per pair via ScalarE-issued DMAs and a Ln/Exp
    reciprocal; RAW attention output + reciprocal rows AllGather to the
    pair peer immediately (latency hides under later rounds); both rank
    blocks are normalized locally (selector-matmul broadcast of 1/D) and
    out-projected one round later into this core's 256 output columns.

DMA streams: bulk traffic and collective recvs on the SP HWDGE,
latency-critical D-row gathers on the ACT HWDGE, collective triggers on
gpsimd (software DGE is ~4us/DMA — never put real transfers there).
"""

import sys

import numpy as np

sys.path.insert(0, "/opt/trn_rl_repo")

DIM = 512
HEADS = 8
DH = 64
INNER = 512
T = 2048
B = 4
EPS = 1e-5
NCORES = 8
HPC = 4  # heads per core
CQ = HPC * DH  # 256 channels per core for each of Q, K, V
NT = T // 128  # 16 token tiles

_cache = {}


def _build_nc(stage=4):
    from contextlib import ExitStack

    import concourse.bacc as bacc
    import concourse.bass as bass
    import concourse.tile as tile
    from concourse import mybir

    f32 = mybir.dt.float32
    bf16 = mybir.dt.bfloat16
    AF = mybir.ActivationFunctionType
    OP = mybir.AluOpType

    # Route every Exp/Ln activation to the one table set that contains both
    # (natural_log_exp_and_others): the default first-match pick splits them
    # across two sets and the per-division ln/exp chain then reloads ACT
    # tables repeatedly (~2.7us each).
    if not getattr(bacc, "_act_tables_patched", False):
        _orig_get_tables = bacc.get_activation_tables

        def _patched_get_tables(arch):
            tabs = _orig_get_tables(arch)
            both = [
                n
                for n, fns in tabs.items()
                if mybir.ActivationFunctionType.Exp in fns
                and mybir.ActivationFunctionType.Ln in fns
            ]
            if both:
                keep = both[0]
                tabs = {
                    n: (
                        fns
                        if n == keep
                        else fns
                        - {
                            mybir.ActivationFunctionType.Exp,
                            mybir.ActivationFunctionType.Ln,
                        }
                    )
                    for n, fns in tabs.items()
                }
            return tabs

        bacc.get_activation_tables = _patched_get_tables
        bacc._act_tables_patched = True

    nc = bacc.Bacc(
        "TRN2",
        target_bir_lowering=False,
        debug=False,
        num_devices=NCORES,
    )

    # ---- external I/O ----
    x_t = nc.dram_tensor("x_t", [DIM, T], bf16, kind="ExternalInput")
    # [512, 768] = [W'_q(256) | W'_k(256) | W'_v(256)] with LN-g folded in,
    # Q/K/V column blocks each ordered [h0|h1|h2|h3] x 64
    w_qkv_s = nc.dram_tensor("w_qkv_s", [DIM, 3 * CQ], bf16, kind="ExternalInput")
    # column sums of the g-folded weights (rank-1 mean correction)
    cs1 = nc.dram_tensor("cs1", [1, 3 * CQ], bf16, kind="ExternalInput")
    # all 512 w_out rows (head order h0..h7 x 64), this core's 256 out-columns
    w_out_s = nc.dram_tensor("w_out_s", [4 * 128, 256], bf16, kind="ExternalInput")
    b_half = nc.dram_tensor("b_half", [1, 256], bf16, kind="ExternalInput")
    # selector for broadcasting the 2 reciprocal rows to 64 partitions each
    sel2 = nc.dram_tensor("sel2", [2, 128], bf16, kind="ExternalInput")
    out_ext = nc.dram_tensor("out", [T, DIM], bf16, kind="ExternalOutput")

    with tile.TileContext(nc) as tc:
        with ExitStack() as stack:
            constp = stack.enter_context(tc.tile_pool(name="const", bufs=1))
            statp = stack.enter_context(tc.tile_pool(name="statp", bufs=2))
            work = stack.enter_context(tc.tile_pool(name="work", bufs=2))
            epool = stack.enter_context(tc.tile_pool(name="epool", bufs=4))
            opool = stack.enter_context(tc.tile_pool(name="opool", bufs=4))
            dpool = stack.enter_context(tc.tile_pool(name="dpool", bufs=4))
            rpool = stack.enter_context(tc.tile_pool(name="rpool", bufs=2))
            gpool = stack.enter_context(tc.tile_pool(name="gpool", bufs=2))
            ps_s = stack.enter_context(tc.tile_pool(name="ps_s", bufs=2, space="PSUM"))
            ps_o = stack.enter_context(tc.tile_pool(name="ps_o", bufs=1, space="PSUM"))
            ps_m = stack.enter_context(tc.tile_pool(name="ps_m", bufs=2, space="PSUM"))
            dram = stack.enter_context(tc.tile_pool(name="dram", bufs=1, space="DRAM"))
            xpool = stack.enter_context(tc.tile_pool(name="xpool", bufs=1))
            pers = stack.enter_context(tc.tile_pool(name="pers", bufs=1))

            # x as 16 fully-contiguous 128KB slabs (strided chunk loads run at
            # ~1/3 bandwidth); everything lands by ~8us
            xw = [
                xpool.tile([128, T], bf16, name=f"x{d}", tag=f"x{d}") for d in range(4)
            ]
            for d in range(4):
                for r in range(4):
                    nc.sync.dma_start(
                        xw[d][32 * r : 32 * (r + 1), :],
                        x_t[d * 128 + 32 * r : d * 128 + 32 * (r + 1), :],
                    )

            # ---------- constants / weights ----------
            w_sb = [
                constp.tile([128, 3 * CQ], bf16, name=f"w{d}", tag=f"w{d}")
                for d in range(4)
            ]
            for d in range(4):
                nc.sync.dma_start(w_sb[d][:], w_qkv_s[d * 128 : (d + 1) * 128, :])
            ones_row = constp.tile([1, 128], bf16)
            nc.vector.memset(ones_row[:], 1.0)
            inv512 = constp.tile([128, 1], bf16)
            nc.vector.memset(inv512[:], 1.0 / DIM)
            eps_col = constp.tile([1, 1], f32)
            nc.vector.memset(eps_col[:], EPS)
            rstd_colb = constp.tile([128, NT], bf16)
            rstd_col = constp.tile([128, NT], f32)

            # needed by the first corr matmul (~10us in): before wout
            csg_sb = constp.tile([1, 3 * CQ], bf16)
            nc.sync.dma_start(csg_sb[:], cs1[:])
            # late-needed weights after the x stream
            wout_sb = [
                constp.tile([128, 256], bf16, name=f"wo{g}", tag=f"wo{g}")
                for g in range(4)
            ]
            for g in range(4):
                nc.sync.dma_start(wout_sb[g][:], w_out_s[g * 128 : (g + 1) * 128, :])
            brep = constp.tile([128, 256], bf16)
            nc.sync.dma_start(brep[:], b_half[:].broadcast_to((128, 256)))
            sel_sb = constp.tile([2, 128], bf16)
            nc.sync.dma_start(sel_sb[:], sel2[:])

            # persistent intermediates
            rstd_rep = pers.tile([128, T], bf16, name="rstd_rep", tag="rstd_rep")
            corr2 = pers.tile([1, T], bf16, name="corr2", tag="corr2")
            qT = [
                pers.tile([128, T], bf16, name=f"qT{p}", tag=f"qT{p}")
                for p in range(2)
            ]
            kT = [
                pers.tile([128, T], bf16, name=f"kT{p}", tag=f"kT{p}")
                for p in range(2)
            ]
            # V per k-tile: 4 blocks of 128 cols; even head block = [V(64) |
            # ones@64 | 0...], odd head block = [0... | ones@32 | 0 | V@64:128]
            v_sb = pers.tile([128, NT * 512], bf16, name="v_sb", tag="v_sb")

            # DRAM bounce tensors
            rstd_dram = dram.tile([1, T], bf16, name="rstd_dram", tag="rstd_dram")
            rstdT_dram = dram.tile([128, NT], bf16, name="rstdT", tag="rstdT")
            # per (pair, chunk) AllGather of RAW attention output + reciprocal
            # rows, fired immediately after each pair's A@V so the collective
            # latency hides under subsequent compute; each core divides the
            # peer's heads locally and out-projects into its own 256 columns
            ag_send = [
                [
                    dram.tile([130, 512], bf16, name=f"as{p}_{j}", tag=f"as{p}_{j}")
                    for j in range(4)
                ]
                for p in range(2)
            ]
            ag_recv = [
                [
                    dram.tile([260, 512], bf16, name=f"ar{p}_{j}", tag=f"ar{p}_{j}")
                    for j in range(4)
                ]
                for p in range(2)
            ]

            # ---------- prologue: LN statistics ----------
            rstd_rows = {}

            def emit_rep(j):
                # partition-broadcast on the PE (ones-row outer product);
                # emitted after ALL stats so the ps_m slot chain never makes
                # a later chunk's stats wait on this chunk's ACT rstd
                # borrow the (still idle) attention S^T PSUM so neither the
                # stats nor the QKV ps_m slot chain couples to the ACT rstd
                cl = slice(j * 512, (j + 1) * 512)
                rep_ps = ps_s.tile([128, 1024], f32, name="rep_ps", tag="s_ps")
                nc.tensor.matmul(rep_ps[:, 0:512], ones_row[:], rstd_rows[j][:])
                nc.vector.tensor_copy(rstd_rep[:, cl], rep_ps[:, 0:512])

            def emit_stats(j):
                cl = slice(j * 512, (j + 1) * 512)
                s1t = ps_m.tile([128, 512], f32, name="s1t", tag="m")
                s2t = ps_m.tile([128, 512], f32, name="s2t", tag="m")
                s1 = s1t[0:1, :]
                s2 = s2t[0:1, :]
                for d in range(4):
                    nc.tensor.matmul(
                        s1, inv512[:], xw[d][:, cl], start=(d == 0), stop=(d == 3)
                    )
                for d in range(4):
                    xsq = work.tile([128, 512], bf16, name="xsq", tag="xsq")
                    nc.vector.tensor_tensor(xsq[:], xw[d][:, cl], xw[d][:, cl], OP.mult)
                    nc.tensor.matmul(
                        s2, inv512[:], xsq[:], start=(d == 0), stop=(d == 3)
                    )
                # corr2 = -mu (rank-1 mean correction row); var from E[x^2]-mu^2
                nc.vector.tensor_scalar(corr2[0:1, cl], s1, -1.0, None, OP.mult)
                musq = statp.tile([1, 512], f32, name="musq", tag="musq")
                nc.vector.tensor_tensor(
                    musq[:], corr2[0:1, cl], corr2[0:1, cl], OP.mult
                )
                var = statp.tile([1, 512], f32, name="var", tag="var")
                nc.vector.tensor_tensor(var[:], s2, musq[:], OP.subtract)
                lnv = statp.tile([1, 512], f32, name="lnv", tag="lnv")
                nc.scalar.activation(lnv[:], var[:], AF.Ln, bias=eps_col[:], scale=1.0)
                rstd_row = statp.tile(
                    [1, 512], bf16, name="rstd_row", tag="rstd_row", bufs=4
                )
                nc.scalar.activation(rstd_row[:], lnv[:], AF.Exp, bias=0.0, scale=-0.5)
                rstd_rows[j] = rstd_row
                # scatter rstd into [token%128, tile] layout for the V scaling
                nc.sync.dma_start(
                    rstdT_dram[:, 4 * j : 4 * j + 4].rearrange("p t -> t p"),
                    rstd_row[:].rearrange("o (t p) -> o t p", p=128),
                )
                nc.sync.dma_start(
                    rstd_colb[:, 4 * j : 4 * j + 4],
                    rstdT_dram[:, 4 * j : 4 * j + 4],
                )

            def emit_rstdcol(j):
                # deferred: this copy waits a 2-hop DMA bounce, so inside the
                # stats chain it would serialize each chunk by ~8us
                nc.vector.tensor_copy(
                    rstd_col[:, 4 * j : 4 * j + 4], rstd_colb[:, 4 * j : 4 * j + 4]
                )

            # ---------- emission helpers ----------
            def emit_qkv_block(j, ct):
                # ct 0,1 -> qT pairs; 2,3 -> kT pairs
                cl = slice(j * 512, (j + 1) * 512)
                dst = qT[ct] if ct < 2 else kT[ct - 2]
                wcl = slice(ct * 128, (ct + 1) * 128)
                acc = ps_m.tile([128, 512], f32, name="qkv_ps", tag="m")
                for d in range(4):
                    nc.tensor.matmul(
                        acc[:],
                        w_sb[d][:, wcl],
                        xw[d][:, cl],
                        start=(d == 0),
                        stop=False,
                    )
                nc.tensor.matmul(
                    acc[:], csg_sb[0:1, wcl], corr2[0:1, cl], start=False, stop=True
                )
                nc.vector.tensor_tensor(dst[:, cl], acc[:], rstd_rep[:, cl], OP.mult)

            def emit_v(tt):
                tl = slice(tt * 128, (tt + 1) * 128)
                vact = ps_m.tile([128, 512], f32, name="v_ps", tag="m")
                vac = vact[:, 0:CQ]
                for d in range(4):
                    nc.tensor.matmul(
                        vac,
                        xw[d][:, tl],
                        w_sb[d][:, 2 * CQ : 3 * CQ],
                        start=(d == 0),
                        stop=False,
                    )
                nc.tensor.matmul(
                    vac,
                    corr2[0:1, tl],
                    csg_sb[0:1, 2 * CQ : 3 * CQ],
                    start=False,
                    stop=True,
                )
                blk = v_sb[:, tt * 512 : (tt + 1) * 512].rearrange(
                    "p (a b) -> p a b", b=256
                )
                src = vac.rearrange("p (a b) -> p a b", b=128)
                rc = rstd_col[:, tt : tt + 1]
                nc.vector.tensor_scalar(
                    blk[:, :, 0:64], src[:, :, 0:64], rc, None, OP.mult
                )
                nc.vector.tensor_scalar(
                    blk[:, :, 192:256], src[:, :, 64:128], rc, None, OP.mult
                )

            def attn_start(j, p):
                # allocate PSUM and emit the first two S^T tiles early so the
                # exp stream restarts as soon as the chunk's qT lands
                nkt = 4 * (j + 1)

                def st(i):
                    q0 = max(512 * j, 128 * i)
                    n = 512 * (j + 1) - q0
                    off = q0 - 512 * j
                    s_ps = ps_s.tile([128, 1024], f32, name="s_ps", tag="s_ps")
                    for h in range(2):
                        hr = slice(h * 64, (h + 1) * 64)
                        nc.tensor.matmul(
                            s_ps[:, h * 512 + off : h * 512 + off + n],
                            kT[p][hr, i * 128 : (i + 1) * 128],
                            qT[p][hr, q0 : q0 + n],
                        )
                    return s_ps, off, n

                o_ps = [
                    ps_o.tile([128, 512], f32, name=f"o_ps{h}", tag=f"o_ps{h}")
                    for h in range(2)
                ]
                ctx = dict(j=j, p=p, st=st, o_ps=o_ps, pend={}, nst=0)

                def more():
                    # pre-emit one more S^T tile (slot aliasing is safe; the
                    # scheduler orders it after the aliased slot's readers)
                    if ctx["nst"] < 4 * (j + 1):
                        ctx["pend"][ctx["nst"]] = st(ctx["nst"])
                        ctx["nst"] += 1

                ctx["more"] = more
                more()
                more()
                return ctx

            def attn_finish(ctx, inject=None):
                j, p, st, o_ps, pend = (
                    ctx["j"],
                    ctx["p"],
                    ctx["st"],
                    ctx["o_ps"],
                    ctx["pend"],
                )
                nkt = 4 * (j + 1)
                for i in range(nkt):
                    s_ps, off, n = pend.pop(i)
                    e_sb = epool.tile([128, 1024], bf16, name="e_sb", tag="e_sb")
                    sr = s_ps[:].rearrange("p (x n) -> p x n", x=2)[:, :, off : off + n]
                    er = e_sb[:].rearrange("p (x n) -> p x n", x=2)[:, :, off : off + n]
                    nc.scalar.activation(er, sr, AF.Exp, bias=0.0, scale=0.125)
                    if stage == 2 and j == 0 and p == 0 and i == 0:
                        nc.sync.dma_start(out_ext[256:384, :], e_sb[:, 0:512])
                    if i == 1 and inject is not None:
                        inject()
                    if i >= 4 * j:
                        # frame-causal quadrant: rows 64:128 (frame 2i+1) must
                        # not be seen by queries 128i..128i+64
                        ez = e_sb[64:128, :].rearrange("p (x n) -> p x n", x=2)[
                            :, :, off : off + 64
                        ]
                        nc.vector.memset(ez, 0.0)
                    first = i == 0
                    last = i == nkt - 1
                    for h in range(2):
                        g = 2 * p + h
                        nc.tensor.matmul(
                            o_ps[h][:, off:512],
                            v_sb[:, i * 512 + g * 128 : i * 512 + (g + 1) * 128],
                            e_sb[:, h * 512 + off : (h + 1) * 512],
                            start=first,
                            stop=last,
                            skip_group_check=True,
                        )
                    while ctx["nst"] < min(i + 3, nkt):
                        ctx["more"]()
                # stash raw O (bf16) and denominator rows; SBUF->SBUF DMA
                # batches both heads' D rows onto partitions 0-1
                oraw = opool.tile([128, 512], bf16, name="oraw", tag="oraw")
                or2 = opool.tile([65, 512], bf16, name="or2", tag="or2")
                nc.vector.tensor_copy(oraw[0:64, :], o_ps[0][0:64, :])
                nc.vector.tensor_copy(oraw[64:128, :], o_ps[1][64:128, :])
                nc.vector.tensor_copy(or2[64:65, :], o_ps[0][64:65, :])
                nc.vector.tensor_copy(or2[32:33, :], o_ps[1][32:33, :])
                dstt = dpool.tile([2, 512], bf16, name="dstt", tag="dstt")
                nc.scalar.dma_start(dstt[0:1, :], or2[64:65, :])
                nc.scalar.dma_start(dstt[1:2, :], or2[32:33, :])
                if stage == 2 and j == 0:
                    nc.sync.dma_start(out_ext[128 * p : 128 * (p + 1), :], oraw[:])
                return oraw, dstt

            def emit_recip(p, j, dst):
                # batched 1/D for both heads: exp(-ln(D))
                lnd = dpool.tile([2, 512], f32, name="lnd", tag="lnd")
                nc.scalar.activation(lnd[:], dst[:], AF.Ln, bias=0.0, scale=1.0)
                rec = dpool.tile([2, 512], bf16, name="rec", tag="rec")
                nc.scalar.activation(rec[:], lnd[:], AF.Exp, bias=0.0, scale=-1.0)
                if stage == 2 and j == 0:
                    nc.sync.dma_start(out_ext[384 + 2 * p : 386 + 2 * p, :], dst[:])
                    nc.sync.dma_start(out_ext[388 + 2 * p : 390 + 2 * p, :], rec[:])
                return rec

            def emit_selmm(rec):
                rrep_ps = ps_m.tile([128, 512], f32, name="rrep_ps", tag="m")
                nc.tensor.matmul(rrep_ps[:], sel_sb[:], rec[:])
                return rrep_ps

            def emit_div(p, j, oraw, rrep_ps):
                cl = slice(j * 512, (j + 1) * 512)
                rrep = rpool.tile([128, 512], bf16, name="rrep", tag="rrep")
                nc.vector.tensor_copy(rrep[:], rrep_ps[:])
                nc.vector.tensor_tensor(onormP[p][:, cl], oraw[:], rrep[:], OP.mult)

            def emit_ag(p, j, oraw, rec):
                # send this pair's raw O + reciprocal rows to the pair peer
                nc.sync.dma_start(ag_send[p][j][0:128, :], oraw[:])
                nc.sync.dma_start(ag_send[p][j][128:130, :], rec[:])
                nc.gpsimd.collective_compute(
                    "AllGather",
                    OP.bypass,
                    replica_groups=[[2 * b, 2 * b + 1] for b in range(B)],
                    ins=[ag_send[p][j][:].opt()],
                    outs=[ag_recv[p][j][:].opt()],
                )
                # AllGather output is rank-ordered, so both rank blocks are
                # fetched and normalized locally (SPMD-uniform)
                out = []
                for r in range(2):
                    po = gpool.tile(
                        [128, 512], bf16, name=f"po{p}{r}", tag=f"po{p}{r}"
                    )
                    pr = gpool.tile([2, 512], bf16, name=f"pc{p}{r}", tag=f"pc{p}{r}")
                    nc.sync.dma_start(po[:], ag_recv[p][j][130 * r : 130 * r + 128])
                    nc.sync.dma_start(
                        pr[:], ag_recv[p][j][130 * r + 128 : 130 * r + 130]
                    )
                    out.append((po, pr))
                return out  # [(rawO, rec) for rank 0, rank 1]

            def emit_norm(po, pr):
                # normalize one gathered head-pair block
                rrep_ps = emit_selmm(pr)
                rrep = rpool.tile([128, 512], bf16, name="prr", tag="prr")
                nc.vector.tensor_copy(rrep[:], rrep_ps[:])
                pn = gpool.tile([128, 512], bf16, name="pn", tag="pn", bufs=4)
                nc.vector.tensor_tensor(pn[:], po[:], rrep[:], OP.mult)
                return pn

            def emit_outproj(tt, pns):
                # pns[g] = normalized [128, 512] chunk block, g = 2*rank + p
                # (natural head order h0..h7); w_out rows natural on all cores
                tl = slice(tt * 128, (tt + 1) * 128)
                lc = slice((tt % 4) * 128, (tt % 4 + 1) * 128)
                ops = ps_m.tile([128, 512], f32, name="out_ps", tag="m")
                for g in range(4):
                    nc.tensor.matmul(
                        ops[:, 0:256],
                        pns[g][:, lc],
                        wout_sb[g][:],
                        start=(g == 0),
                        stop=(g == 3),
                    )
                pst = work.tile([128, 256], bf16, name="pst", tag="pst")
                nc.vector.tensor_tensor(pst[:], ops[:, 0:256], brep[:], OP.add)
                nc.sync.dma_start(out_ext[tl, 0:256], pst[:])

            # ---------- pipelined rounds ----------
            # QKV for chunk j+1 is emitted inside round j, so each round's
            # attention starts on data prepared one round earlier
            agst = {}  # (p, j) -> [(rawO, rec) per rank]

            def emit_norms(jj):
                # normalize chunk jj's gathered blocks (g = 2*rank + p)
                ag0, ag1 = agst.pop((0, jj)), agst.pop((1, jj))
                pns = [
                    emit_norm(*ag0[0]),
                    emit_norm(*ag1[0]),
                    emit_norm(*ag0[1]),
                    emit_norm(*ag1[1]),
                ]
                if stage == 3 and jj in (0, 3):
                    ro = 0 if jj == 0 else 256
                    nc.sync.dma_start(out_ext[ro : ro + 128, :], pns[0][:])
                    nc.sync.dma_start(out_ext[ro + 128 : ro + 256, :], pns[1][:])
                return pns

            # prologue: only what round-0 attention needs (chunks 2/3 stats
            # would head-block the PE on their x DMAs; they move to round 0)
            emit_stats(0)
            emit_stats(1)
            emit_stats(2)
            emit_stats(3)
            for j in range(4):
                emit_rep(j)
            emit_qkv_block(0, 0)
            emit_qkv_block(0, 2)
            emit_qkv_block(0, 1)
            emit_qkv_block(0, 3)
            for j in range(4):
                emit_rstdcol(j)
            # V staging memsets precede the first emit_v in DVE order
            nc.vector.memset(v_sb[:], 0.0)
            vview = v_sb[:].rearrange("p (t a b) -> p t a b", t=NT, b=256)
            nc.vector.memset(vview[:, :, :, 64:65], 1.0)
            nc.vector.memset(vview[:, :, :, 160:161], 1.0)
            for tt in range(4):
                emit_v(tt)
            a0 = attn_start(0, 0) if stage >= 2 else None
            for j in range(4):
                if stage >= 2:
                    oraw0, dst0 = attn_finish(a0)

                    def inj1(jj=j, o=oraw0, d=dst0):
                        rec = emit_recip(0, jj, d)
                        if stage >= 3:
                            agst[(0, jj)] = emit_ag(0, jj, o, rec)

                    # p1's S^T pipeline starts before next chunk's QKV so the
                    # exp stream never waits behind the QKV matmul block
                    a1 = attn_start(j, 1)
                    if j < 3:
                        emit_qkv_block(j + 1, 0)
                        a1["more"]()
                        emit_qkv_block(j + 1, 2)
                        a1["more"]()
                    oraw1, dst1 = attn_finish(a1, inject=inj1)
                    if j < 3:
                        emit_qkv_block(j + 1, 1)
                        emit_qkv_block(j + 1, 3)
                        for tt in range(4 * (j + 1), 4 * (j + 1) + 4):
                            emit_v(tt)
                    rec1 = emit_recip(1, j, dst1)
                    if stage >= 3:
                        agst[(1, j)] = emit_ag(1, j, oraw1, rec1)
                    if j < 3:
                        a0 = attn_start(j + 1, 0)
                    if j > 0 and stage >= 3:
                        pns = emit_norms(j - 1)
                        if stage >= 4:
                            for tt in range(4 * (j - 1), 4 * j):
                                emit_outproj(tt, pns)
                elif j < 3:
                    emit_qkv_block(j + 1, 0)
                    emit_qkv_block(j + 1, 2)
                    emit_qkv_block(j + 1, 1)
                    emit_qkv_block(j + 1, 3)
                    for tt in range(4 * (j + 1), 4 * (j + 1) + 4):
                        emit_v(tt)
            # tail: last chunk's gathers, normalize, out-proj
            if stage >= 3:
                pns = emit_norms(3)
            if stage >= 4:
                for tt in range(12, 16):
                    emit_outproj(tt, pns)

            # ---------- debug dumps ----------
            if stage == 1:
                nc.sync.dma_start(out_ext[0:128, :], qT[0][:, 0:512])
                nc.sync.dma_start(out_ext[128:256, :], kT[0][:, 0:512])
                nc.sync.dma_start(out_ext[256:384, :], v_sb[:, 0:512])
                nc.sync.dma_start(out_ext[384:512, :], rstd_rep[:, 0:512])
            if stage == 2:
                nc.sync.dma_start(out_ext[400:528, :], v_sb[:, 1024:1536])
                nc.sync.dma_start(out_ext[528:656, :], v_sb[:, 1536:2048])
                nc.sync.dma_start(out_ext[656:784, 0:NT], rstd_colb[:])
            if stage == 3:
                nc.sync.dma_start(out_ext[0:128, :], onormP[0][:, 0:512])
                nc.sync.dma_start(out_ext[128:256, :], onormP[1][:, 0:512])
                nc.sync.dma_start(out_ext[256:384, :], onormP[0][:, 1536:2048])
                nc.sync.dma_start(out_ext[384:512, :], onormP[1][:, 1536:2048])

    nc.compile()
    return nc


def _prep_in_maps(x, ln_g, ln_b, w_qkv, w_out, b_out):
    import ml_dtypes

    bf = ml_dtypes.bfloat16
    wp = ln_g[:, None] * w_qkv  # [512, 1536]
    csb = (ln_b[:, None] * w_qkv).sum(axis=0)
    assert np.abs(csb).max() == 0.0, "nonzero ln_b not supported by this build"
    sel = np.zeros((2, 128), dtype=np.float32)
    sel[0, :64] = 1.0
    sel[1, 64:] = 1.0
    in_maps = []
    for c in range(NCORES):
        b = c // 2
        heads = range(4 * (c % 2), 4 * (c % 2) + 4)
        qcols = np.concatenate([np.arange(h * DH, (h + 1) * DH) for h in heads])
        cols = np.concatenate([qcols, INNER + qcols, 2 * INNER + qcols])
        ocols = slice(0, 256) if c % 2 == 0 else slice(256, 512)
        in_maps.append(
            {
                "x_t": np.ascontiguousarray(x[b].T).astype(bf),
                "w_qkv_s": np.ascontiguousarray(wp[:, cols]).astype(bf),
                "cs1": np.ascontiguousarray(wp[:, cols].sum(axis=0))
                .reshape(1, -1)
                .astype(bf),
                "w_out_s": np.ascontiguousarray(w_out[:, ocols]).astype(bf),
                "b_half": np.ascontiguousarray(b_out[ocols]).reshape(1, 256).astype(bf),
                "sel2": sel.astype(bf),
            }
        )
    return in_maps


def _run(inputs, trace=False):
    from concourse.bass_utils import run_bass_kernel_spmd

    import os

    stage = int(os.environ.get("KSTAGE", "4"))
    if ("nc", stage) not in _cache:
        _cache[("nc", stage)] = _build_nc(stage)
    nc = _cache[("nc", stage)]
    in_maps = _prep_in_maps(
        np.asarray(inputs["x"], dtype=np.float32),
        np.asarray(inputs["ln_g"], dtype=np.float32),
        np.asarray(inputs["ln_b"], dtype=np.float32),
        np.asarray(inputs["w_qkv"], dtype=np.float32),
        np.asarray(inputs["w_out"], dtype=np.float32),
        np.asarray(inputs["b_out"], dtype=np.float32),
    )
    res = run_bass_kernel_spmd(nc, in_maps, core_ids=list(range(NCORES)), trace=trace)
    out = np.empty((B, T, DIM), dtype=np.float32)
    for b in range(B):
        ev = res.results[2 * b]["out"].astype(np.float32)
        od = res.results[2 * b + 1]["out"].astype(np.float32)
        if stage >= 4:
            # each core computed its own 256 output columns (into cols 0:256)
            out[b][:, 0:256] = ev[:, 0:256]
            out[b][:, 256:512] = od[:, 0:256]
        else:
            out[b] = ev
    return out, res


def kernel(**inputs):
    return _run(inputs, trace=False)[0]


def kernel_traced(**inputs):
    out, res = _run(inputs, trace=True)
    return out, res
